# revision 22
# baseline (speedup 1.0000x reference)
"""RGCN-BDD link-predict layer kernel for 8 TRN2 NeuronCores.

Strategy: shard edges by destination-node slice (6250 nodes/device) so the
segment-sum is fully local; run the two RGCN layers as two launches of one
compiled single-layer NEFF, with host-side ReLU between launches (bias is
baked into an extra self-loop weight row fed by a constant-1 xtp row).

Layout tricks:
  - edge slots are chunk-aligned ACROSS devices (per-chunk offsets are the
    cumulative max edge count over devices), so the per-chunk edge-tile
    windows are identical on every core with no union inflation.
  - src features are host-permuted to i-major [i, b] column order and the
    per-edge weight rows to [i, j, b], so one full-width DVE multiply with
    a stride-1-innermost broadcast view forms all 2500 partial products
    (no expansion op, DVE 2x perf mode preserved).
  - i-slice folding is split between DVE pairwise adds (including a fused
    two-pair add) and PE accumulating matmuls, per-tile pattern NADDS.
  - one-hot segment-sum matrices (entries carry edge norm) are fp8.
"""
import sys
if '/opt/trn_rl_repo' not in sys.path:
    sys.path.insert(0, '/opt/trn_rl_repo')

import numpy as np
import ml_dtypes

import concourse.bass as bass
import concourse.bacc as bacc
import concourse.mybir as mybir
import concourse.tile as tile
from concourse.bass_utils import run_bass_kernel_spmd

# problem constants (hardcoded per spec)
NN = 50000      # num nodes
H = 500         # hidden dim
NB = 100        # num bases
SUB = 5         # block size
W_COLS = NB * SUB * SUB  # 2500
NR2 = 474       # num relations * 2
E = 100000      # num edges
NDEV = 8
P = 128
NPD = NN // NDEV          # 6250 nodes per device
NCH = (NPD + P - 1) // P  # 49 chunks
N_PAD = NCH * P           # 6272
KQ4 = 512  # K padded to 4*128 (zero rows beyond 500; row 500 = bias)

# feature-column permutation used for both the i-major x layout and the
# (j, b)-ordered output columns: PERM[v*NB + b] = b*SUB + v
PERM = np.array([b * SUB + v for v in range(SUB) for b in range(NB)])

WG_FP8 = True    # weight table stored fp8 in DRAM, cast to bf16 during gather
OH_FP8 = True    # one-hot matrices in fp8 (PE lhsT)


def nadd_for(t):
    """i-slices folded on DVE for edge tile t (rest go to PE as matmuls)."""
    return 1


BF = mybir.dt.bfloat16
F8 = mybir.dt.float8e4
F32 = mybir.dt.float32
I32 = mybir.dt.int32

_cache = {}


def _plan(src, dst, etype, norm):
    """Host-side sharding plan; layer-invariant. Chunk-aligned slot layout."""
    src = np.asarray(src).astype(np.int64)
    dst = np.asarray(dst).astype(np.int64)
    etype = np.asarray(etype).astype(np.int64)
    norm = np.asarray(norm).astype(np.float32).reshape(-1)

    dev_of = dst // NPD
    # per-device edges grouped by chunk
    per = []   # list of (edge_idx_sorted_by_dst, local_dst)
    cnt = np.zeros((NDEV, NCH), np.int64)
    for d in range(NDEV):
        sel = np.nonzero(dev_of == d)[0]
        dl = dst[sel] - d * NPD
        order = np.argsort(dl, kind='stable')
        per.append((sel[order], dl[order]))
        cnt[d] = np.bincount(dl // P, minlength=NCH)
    m = cnt.max(axis=0)                      # per-chunk slot count (shared)
    off = np.concatenate([[0], np.cumsum(m)])
    ET = int(np.ceil(off[-1] / P))
    W0 = off[:-1] // P
    WEND = np.ceil(off[1:] / P).astype(np.int64)
    KE = (WEND - W0).astype(np.int64)
    OHT = int(KE.sum())
    ohoff = np.concatenate([[0], np.cumsum(KE)])[:NCH].astype(np.int64)

    srcl = np.zeros((NDEV, ET * P), np.int64)
    etn = np.zeros((NDEV, P, ET), np.int32)
    oh = np.zeros((NDEV, OHT * P, P), np.float32)
    for d in range(NDEV):
        el, dl = per[d]
        nr = norm[el]
        e0 = 0
        for c in range(NCH):
            n_dc = int(cnt[d, c])
            q = np.arange(n_dc)
            s = off[c] + q                   # global slots for these edges
            srcl[d][s] = src[el[e0:e0 + n_dc]]
            etn[d][s % P, s // P] = etype[el[e0:e0 + n_dc]]
            kk = s // P - W0[c]
            mcol = dl[e0:e0 + n_dc] - c * P
            oh[d, (ohoff[c] + kk) * P + (s % P), mcol] = nr[e0:e0 + n_dc]
            e0 += n_dc

    oh_dt = ml_dtypes.float8_e4m3 if OH_FP8 else ml_dtypes.bfloat16
    return dict(ET=ET, srcl=srcl, etn=etn, oh=oh.astype(oh_dt),
                W0=W0, KE=KE, ohoff=ohoff, OHT=OHT, KEMAX=int(KE.max()))


def _build_nc(ET, W0, KE, ohoff, OHT, KEMAX):
    nc = bacc.Bacc(None, target_bir_lowering=False)

    wg_dt = F8 if WG_FP8 else BF
    oh_dt = F8 if OH_FP8 else BF
    xs = nc.dram_tensor("xs", [ET * P, H], BF, kind="ExternalInput")
    xtp = nc.dram_tensor("xtp", [P, 4, N_PAD], BF, kind="ExternalInput")
    wf = nc.dram_tensor("wf", [NR2, W_COLS], wg_dt, kind="ExternalInput")
    lw = nc.dram_tensor("lw", [KQ4, H], BF, kind="ExternalInput")
    etn = nc.dram_tensor("etn", [P, ET], I32, kind="ExternalInput")
    oh = nc.dram_tensor("oh", [OHT * P, P], oh_dt, kind="ExternalInput")
    out = nc.dram_tensor("out", [N_PAD, H], BF, kind="ExternalOutput")

    with tile.TileContext(nc) as tc:
        with tc.tile_pool(name="const", bufs=1) as constp, \
             tc.tile_pool(name="s1", bufs=4) as s1, \
             tc.tile_pool(name="prodp", bufs=7) as prodp, \
             tc.tile_pool(name="s2", bufs=4) as s2, \
             tc.tile_pool(name="psum", bufs=6, space="PSUM") as psp:

            # etn first: it gates every weight gather
            etn_sb = constp.tile([P, ET], I32, tag="etn")
            nc.sync.dma_start(out=etn_sb[:], in_=etn[:, :])
            lw_sb = []
            for q in range(4):
                t = constp.tile([P, H], BF, tag=f"lw{q}")
                nc.sync.dma_start(out=t[:], in_=lw[q * 128:(q + 1) * 128, :])
                lw_sb.append(t)

            prods = {}   # edge-tile idx -> list of rhs views
            pair_done = set()

            def produce(t):
                if t in pair_done:
                    return
                # process edge tiles in pairs: one xs DMA (scalar HWDGE
                # queue), two single-row-indexed weight gathers into one
                # tile, ONE wide DVE multiply covering both tiles
                tb = t - (t % 2)
                k = 2 if tb + 1 < ET else 1
                xe2 = s1.tile([P, 2, H], BF, tag="xe")
                # all prefetchable loads go on the sync HWDGE queue; the
                # queue is FIFO per engine, so compute-dependent DMAs (the
                # out writes) must NOT share it or they block prefetch
                nc.sync.dma_start(
                    out=xe2[:, :k, :],
                    in_=xs[tb * P:(tb + k) * P, :].rearrange(
                        "(k p) h -> p k h", p=P))
                wgp = s1.tile([P, 2, W_COLS], BF, tag="wg")
                for kk in range(k):
                    nc.gpsimd.indirect_dma_start(
                        out=wgp[:, kk, :], out_offset=None, in_=wf[:, :],
                        in_offset=bass.IndirectOffsetOnAxis(
                            ap=etn_sb[:, tb + kk:tb + kk + 1], axis=0))
                # all partial products for the pair in one DVE op (2x perf
                # mode): xe columns i-major [i, b]; wg rows [i, j, b]
                prod2 = prodp.tile([P, 2, W_COLS], BF, tag="prod")
                xe_b = xe2[:, :k, :].rearrange(
                    "p k (i u b) -> p k i u b", i=SUB, u=1) \
                    .to_broadcast([P, k, SUB, SUB, NB])
                nc.vector.tensor_tensor(
                    out=prod2[:, :k, :].rearrange(
                        "p k (i j b) -> p k i j b", i=SUB, j=SUB),
                    in0=xe_b,
                    in1=wgp[:, :k, :].rearrange(
                        "p k (i j b) -> p k i j b", i=SUB, j=SUB),
                    op=mybir.AluOpType.mult)
                for kk in range(k):
                    tt = tb + kk
                    # fold i-slices: DVE pairwise adds; remaining slices are
                    # summed by PE accumulating matmuls
                    nadd = nadd_for(tt)
                    sl = [prod2[:, kk, i * H:(i + 1) * H] for i in range(SUB)]
                    if nadd == 1:
                        s01 = prodp.tile([P, H], BF, tag="s01")
                        nc.vector.tensor_tensor(out=s01[:], in0=sl[0],
                                                in1=sl[1],
                                                op=mybir.AluOpType.add)
                        sl = [s01[:]] + sl[2:]
                    elif nadd >= 2:
                        # one op: (p0||p2) + (p1||p3) -> (p0+p1 || p2+p3)
                        s0123 = prodp.tile([P, 2 * H], BF, tag="s0123")
                        base = prod2[:, kk, :]
                        v02 = base[:, 0:4 * H].rearrange(
                            "p (x f) -> p x f", x=2)[:, :, 0:H]
                        v13 = base[:, H:5 * H].rearrange(
                            "p (x f) -> p x f", x=2)[:, :, 0:H]
                        nc.vector.tensor_tensor(
                            out=s0123[:].rearrange("p (x f) -> p x f", x=2),
                            in0=v02, in1=v13, op=mybir.AluOpType.add)
                        if nadd >= 3:
                            s03 = prodp.tile([P, H], BF, tag="s03")
                            nc.vector.tensor_tensor(
                                out=s03[:], in0=s0123[:, 0:H],
                                in1=s0123[:, H:2 * H],
                                op=mybir.AluOpType.add)
                            sl = [s03[:], sl[4]]
                        else:
                            sl = [s0123[:, 0:H], s0123[:, H:2 * H], sl[4]]
                    prods[tt] = sl
                    pair_done.add(tt)

            produced = 0
            for c in range(NCH):
                need = int(W0[c] + KE[c])
                while produced < need:
                    produce(produced)
                    produced += 1
                ps = psp.tile([P, H], F32, tag="ps")
                ke = int(KE[c])
                ohsb = s2.tile([P, KEMAX * P], oh_dt, tag="ohsb")
                o0 = int(ohoff[c]) * P
                nc.sync.dma_start(
                    out=ohsb[:, :ke * P].rearrange("p (k m) -> p k m", k=ke),
                    in_=oh[o0:o0 + ke * P, :].rearrange("(k p) m -> p k m", p=P))
                xt = s2.tile([P, 4, P], BF, tag="xt")
                nc.sync.dma_start(out=xt[:], in_=xtp[:, :, c * P:(c + 1) * P])
                first = True
                for kk in range(ke):
                    t = int(W0[c]) + kk
                    for rv in prods[t]:
                        nc.tensor.matmul(out=ps[:],
                                         lhsT=ohsb[:, kk * P:(kk + 1) * P],
                                         rhs=rv, start=first, stop=False)
                        first = False
                for q in range(4):
                    nc.tensor.matmul(out=ps[:], lhsT=xt[:, q, :],
                                     rhs=lw_sb[q][:],
                                     start=False, stop=(q == 3))
                outt = s2.tile([P, H], BF, tag="outt")
                nc.scalar.activation(out=outt[:], in_=ps[:],
                                     func=mybir.ActivationFunctionType.Copy)
                # out write rides the scalar queue: it only ever waits on its
                # own ACT copy, so no prefetch sits behind it
                nc.scalar.dma_start(out=out[c * P:(c + 1) * P, :], in_=outt[:])
                if c + 1 < NCH:
                    for t in [k for k in prods if k < int(W0[c + 1])]:
                        del prods[t]
    nc.finalize()
    return nc


def _run_layer(nc, plan, x, wfp, lwb, trace=False):
    """One RGCN-BDD layer (incl. bias, pre-activation) on 8 cores."""
    xb = x.astype(ml_dtypes.bfloat16)
    xp = np.ascontiguousarray(xb[:, PERM])   # i-major columns for messages
    in_maps = []
    for d in range(NDEV):
        xsd = np.ascontiguousarray(xp[plan['srcl'][d]])
        xtpd = np.zeros((P, 4, N_PAD), ml_dtypes.bfloat16)
        xst = xb[d * NPD:(d + 1) * NPD].T  # [500, NPD], original column order
        for q in range(4):
            rows = min(128, H - q * 128)
            xtpd[:rows, q, :NPD] = xst[q * 128:q * 128 + rows]
        xtpd[116, 3, :NPD] = 1.0  # constant-1 row feeding the bias row of lw
        in_maps.append({
            "xs": xsd, "xtp": np.ascontiguousarray(xtpd), "wf": wfp, "lw": lwb,
            "etn": plan['etn'][d], "oh": plan['oh'][d],
        })
    res = run_bass_kernel_spmd(nc, in_maps, core_ids=list(range(NDEV)),
                               trace=trace)
    outp = np.empty((NN, H), np.float32)
    for d in range(NDEV):
        # device columns are in (j, b) order; un-permute back to (b, j)
        outp[d * NPD:(d + 1) * NPD][:, PERM] = \
            res.results[d]["out"][:NPD].astype(np.float32)
    return outp, res


def _prep_lw(lw, bias):
    lwp = np.zeros((KQ4, H), np.float32)
    lwp[:H] = np.asarray(lw, np.float32)[:, PERM]
    lwp[H] = np.asarray(bias, np.float32)[PERM]
    return lwp.astype(ml_dtypes.bfloat16)


def _permute_w(W):
    # [r, b, i, j] -> [r, i, j, b] flattened
    W = np.asarray(W, dtype=np.float32).reshape(NR2, NB, SUB, SUB)
    Wp = np.ascontiguousarray(W.transpose(0, 2, 3, 1).reshape(NR2, W_COLS))
    return Wp.astype(ml_dtypes.float8_e4m3 if WG_FP8 else ml_dtypes.bfloat16)


def kernel(nids, src, dst, etype, norm, emb, W1, loop_w1, bias1,
           W2, loop_w2, bias2, _trace=False, _times=None):
    import hashlib
    key = hashlib.blake2b(
        np.ascontiguousarray(np.asarray(src, np.int64)).tobytes() +
        np.ascontiguousarray(np.asarray(dst, np.int64)).tobytes() +
        np.ascontiguousarray(np.asarray(etype, np.int64)).tobytes() +
        np.ascontiguousarray(np.asarray(norm, np.float32)).tobytes(),
        digest_size=16).hexdigest()
    if key not in _cache:
        _cache.clear()
        plan = _plan(src, dst, etype, norm)
        nc = _build_nc(plan['ET'], plan['W0'], plan['KE'],
                       plan['ohoff'], plan['OHT'], plan['KEMAX'])
        _cache[key] = (plan, nc)
    plan, nc = _cache[key]

    x = np.asarray(emb, dtype=np.float32)[np.asarray(nids, dtype=np.int64)]
    h_pre, r1 = _run_layer(nc, plan, x, _permute_w(W1),
                           _prep_lw(loop_w1, bias1), trace=_trace)
    h = np.maximum(h_pre, 0.0)
    out, r2 = _run_layer(nc, plan, h, _permute_w(W2),
                         _prep_lw(loop_w2, bias2), trace=_trace)
    if _times is not None:
        _times.extend([r1, r2])
    return out
